# revision 30
# baseline (speedup 1.0000x reference)
"""Trainium2 Bass kernel for 16-head causal MultiHeadAttention.

Problem shapes (hardcoded): x [4, 2048, 1024], Wq/Wk/Wv [1024, 1024],
Wc [1024, 1024], bc [1024].  Output [4, 2048, 1024].

Sharding: 8 cores = (batch b in 0..3) x (head-group g in 0..1).
Each core computes 8 heads (512 of the 1024 hidden dims) for one batch
element, including its partial c_proj contribution.  The host sums the
two partials per batch and adds the bias.

Per-core kernel (all matmuls fp32r = full PE rate):
  P1:  one pass over x^T computing Q^T, K^T = W^T @ x_b^T [512, 2048]
       and V = x_b @ Wv_g (stored with a ones column per head)
  P2:  per (head, q-chunk): S^T = K @ Q^T tiles -> exp (scale 1/8,
       causal mask) -> O^T/denominator accumulate via [V | 1] stationary
       -> normalize rows by 1/denominator (DRAM-bounce broadcast of the
       denominator + fast reciprocal, off the PSUM critical path)
  P3:  partial out = O @ Wc_g   (O^T chunks are the matmul stationaries)
"""

import numpy as np

B, T, C = 4, 2048, 1024
H_PER_CORE = 8       # heads per core
HL = 512             # local head width  (8 heads * 64)
D = 64               # head dim
QC = 512             # q-chunk width (moving free dim)
NQC = T // QC        # 4
NKC = T // 128       # 16
N_CORES = 8

_CACHE = {}


def _emit(nc, tc, tile, mybir, io):
    import concourse.bass as bass
    f32, bf16 = mybir.dt.float32, mybir.dt.bfloat16
    Exp = mybir.ActivationFunctionType.Exp
    xT, wq, wk, wv, wc, maskw, out = (
        io["xT"], io["wq"], io["wk"], io["wv"], io["wc"],
        io["maskw"], io["out"],
    )

    from contextlib import ExitStack

    with ExitStack() as ctx:
        persist = ctx.enter_context(tc.tile_pool(name="persist", bufs=1))
        # Q^T / K^T / O^T: [512 rows, 2048 toks] as [128, 4 chunks, 2048]
        qt = persist.tile([128, 4, T], bf16)
        kt = persist.tile([128, 4, T], bf16)
        # V': [2048 toks, 8 heads x (64 dims + ones col)] as [128, 16, 520]
        vp = persist.tile([128, NKC, H_PER_CORE * (D + 1)], bf16)
        vp4 = vp.rearrange("p mt (h c) -> p mt h c", c=D + 1)
        # ones column per head (denominator accumulator in the O matmuls)
        nc.gpsimd.memset(vp4[:, :, :, D], 1.0)

        # ------- Phase 1: Q^T, K^T, V in one pass over x^T -------
        with (
            tc.tile_pool(name="wqk", bufs=1) as wpool,
            tc.tile_pool(name="xtp", bufs=16) as xtp,
            tc.tile_pool(name="ps1", bufs=6, space="PSUM") as ps1,
        ):
            wq_t = [wpool.tile([128, HL], bf16, tag=f"wq{kc}", name=f"wq{kc}")
                    for kc in range(8)]
            wk_t = [wpool.tile([128, HL], bf16, tag=f"wk{kc}", name=f"wk{kc}")
                    for kc in range(8)]
            wv_t = [wpool.tile([128, HL], bf16, tag=f"wv{kc}", name=f"wv{kc}")
                    for kc in range(8)]

            def load_xt(n, with_weights=False):
                """x^T [1024, 512-tok chunk n] as 8 per-kc tiles; on the
                first chunk the weight loads ride interleaved so matmul kc
                only waits for its own kc-slice of x and W."""
                xts = []
                for kc in range(8):
                    t = xtp.tile([128, QC], bf16, tag="xt")
                    nc.sync.dma_start(
                        out=t, in_=xT[kc * 128:(kc + 1) * 128,
                                      n * QC:(n + 1) * QC])
                    if with_weights:
                        nc.sync.dma_start(
                            out=wq_t[kc], in_=wq[kc * 128:(kc + 1) * 128, :])
                        nc.sync.dma_start(
                            out=wk_t[kc], in_=wk[kc * 128:(kc + 1) * 128, :])
                        nc.sync.dma_start(
                            out=wv_t[kc], in_=wv[kc * 128:(kc + 1) * 128, :])
                    xts.append(t)
                return xts

            for n in range(NQC):
                xs = load_xt(n, with_weights=(n == 0))
                for mc in range(4):
                    pq = ps1.tile([128, QC], f32, tag="p1")
                    for kc in range(8):
                        nc.tensor.matmul(
                            out=pq[:], lhsT=wq_t[kc][:, mc * 128:(mc + 1) * 128],
                            rhs=xs[kc], start=(kc == 0), stop=(kc == 7))
                    nc.scalar.copy(qt[:, mc, n * QC:(n + 1) * QC], pq[:])
                    pk = ps1.tile([128, QC], f32, tag="p1")
                    for kc in range(8):
                        nc.tensor.matmul(
                            out=pk[:], lhsT=wk_t[kc][:, mc * 128:(mc + 1) * 128],
                            rhs=xs[kc], start=(kc == 0), stop=(kc == 7))
                    nc.vector.tensor_copy(kt[:, mc, n * QC:(n + 1) * QC], pk[:])
                for mt in range(4):
                    gm = n * 4 + mt           # global token chunk (0..15)
                    pv = ps1.tile([128, HL], f32, tag="p1")
                    for kc in range(8):
                        nc.tensor.matmul(
                            out=pv[:], lhsT=xs[kc][:, mt * 128:(mt + 1) * 128],
                            rhs=wv_t[kc], start=(kc == 0), stop=(kc == 7))
                    nc.vector.tensor_copy(
                        vp4[:, gm, :, 0:D],
                        pv.rearrange("p (h d) -> p h d", d=D))

        # ---------------- Phase 2: attention ----------------
        with tc.tile_pool(name="otp", bufs=1) as otpool, \
             tc.tile_pool(name="wcp", bufs=1) as wcpool:
            ot = otpool.tile([128, 4, T], bf16)
            # preload Wc during P2 (used in P3)
            wc_sb = wcpool.tile([128, 4, C], bf16)
            nc.sync.dma_start(
                out=wc_sb, in_=wc.rearrange("(kd p) m -> p kd m", p=128))

            with (
                tc.tile_pool(name="mk", bufs=1) as mkpool,
                tc.tile_pool(name="etp", bufs=6) as etp,
                tc.tile_pool(name="smp", bufs=4) as smp,
                tc.tile_pool(name="drp", bufs=8, space="DRAM") as drp,
                tc.tile_pool(name="psw", bufs=2, space="PSUM") as psw,
                tc.tile_pool(name="pso", bufs=2, space="PSUM") as pso,
                tc.tile_pool(name="stp", bufs=4) as stp,
                tc.tile_pool(name="ps3", bufs=2, space="PSUM") as ps3,
            ):
                # causal triangle for the 128-wide diagonal block,
                # duplicated so one tensor_mul masks a head pair at once
                mask_sb = mkpool.tile([128, 2, 128], bf16)
                nc.sync.dma_start(out=mask_sb, in_=maskw)

                def emit_chunk(ha, hb, qc, po_a, po_b, kc, K):
                    """One k-chunk for a head pair: adjacent 64-row S-mms
                    (PE-tile paired), joint exp + mask, two O-mms.

                    Diagonal chunks (d >= 0) skip the fully-masked leading
                    q-columns: only q >= 128*d can attend to this chunk, so
                    the S/exp/O work all shrink; just the leading 128-wide
                    block needs the causal triangle mask."""
                    d = kc - 4 * qc
                    off = max(d, 0) * 128
                    N = QC - off
                    pw = psw.tile([128, 2, QC], f32, tag="pw")
                    for j, h in ((0, ha), (1, hb)):
                        r0 = (h % 2) * 64
                        chh = h // 2
                        # 64-row array tiling: even heads use PE rows 0-63,
                        # odd heads rows 64-127 — the two adjacent S-matmuls
                        # run concurrently on the two halves.
                        nc.tensor.matmul(
                            out=pw[:, j, 0:N],
                            lhsT=kt[r0:r0 + 64, chh, kc * 128:(kc + 1) * 128],
                            rhs=qt[r0:r0 + 64, chh,
                                   qc * QC + off:(qc + 1) * QC],
                            start=True, stop=True, tile_position=(r0, 0))
                    ew = etp.tile([128, 2, QC], bf16, tag="et")
                    nc.scalar.activation(ew[:, :, 0:N], pw[:, :, 0:N],
                                         Exp, scale=0.125)
                    if d >= 0:               # diagonal block: causal triangle
                        nc.vector.tensor_mul(
                            ew[:, :, 0:128], ew[:, :, 0:128], mask_sb[:])
                    for j, h, po in ((0, ha, po_a), (1, hb, po_b)):
                        nc.tensor.matmul(
                            out=po[0:D + 1, off:QC],
                            lhsT=vp[:, kc, h * (D + 1):(h + 1) * (D + 1)],
                            rhs=ew[:, j, 0:N],
                            start=(kc == 0), stop=(kc == K - 1))

                def evict(h, qc, po):
                    """PSUM eviction with a single po read (frees the PSUM
                    bank after one op) + off-critical-path normalization.

                    Reciprocal runs on the single denominator row, then the
                    bf16 reciprocal is broadcast via a tiny DRAM bounce
                    (1 KB) instead of broadcasting the raw denominator and
                    computing 128 reciprocal rows."""
                    r0 = (h % 2) * 64
                    chh = h // 2
                    ot_slice = ot[r0:r0 + 64, chh, qc * QC:(qc + 1) * QC]
                    og = smp.tile([65, QC], f32, tag="og")
                    nc.vector.tensor_copy(og[:], po[0:D + 1, :])
                    d1 = smp.tile([1, QC], f32, tag="d1")
                    nc.vector.tensor_copy(d1[:], og[D:D + 1, :])
                    nc.vector.reciprocal_approx_fast(d1[:], d1[:])
                    dr = smp.tile([1, QC], bf16, tag="dr")
                    nc.vector.tensor_copy(dr[:], d1[:])
                    scr = drp.tile([1, QC], bf16, tag="scr")
                    nc.sync.dma_start(out=scr[:], in_=dr[:])
                    db = smp.tile([64, QC], bf16, tag="db")
                    s0 = scr[:]
                    nc.gpsimd.dma_start(
                        out=db[:],
                        in_=bass.AP(tensor=s0.tensor, offset=s0.offset,
                                    ap=[[0, 64], [1, QC]]))
                    nc.vector.tensor_mul(ot_slice, og[0:64, :], db[:])

                def c_proj(mt):
                    """c_proj partial for one 128-token chunk (interleaved
                    into P2 so its exp-independent matmuls fill PE bubbles)."""
                    for n2 in range(2):
                        pc = ps3.tile([128, QC], f32, tag="pc")
                        for kd in range(4):
                            nc.tensor.matmul(
                                out=pc[:],
                                lhsT=ot[:, kd, mt * 128:(mt + 1) * 128],
                                rhs=wc_sb[:, kd, n2 * QC:(n2 + 1) * QC],
                                start=(kd == 0), stop=(kd == 3))
                        st = stp.tile([128, QC], bf16, tag="st")
                        if (mt + n2) % 2 == 0:
                            nc.vector.tensor_copy(st[:], pc[:])
                        else:
                            nc.scalar.copy(st[:], pc[:])
                        nc.sync.dma_start(
                            out=out[mt * 128:(mt + 1) * 128,
                                    n2 * QC:(n2 + 1) * QC],
                            in_=st[:])

                # qc-outer / head-pair-inner: once all 4 pairs finished a
                # q-chunk, its tokens' c_proj runs right away, overlapping
                # the next q-chunk's attention.  Head pairs in chunk
                # lockstep so the two heads' 64-row S-matmuls are adjacent
                # and fill both PE array halves.
                for qc in range(NQC):
                    K = 4 * qc + 4          # causal k-chunks for this q-chunk
                    for hp in range(H_PER_CORE // 2):
                        ha, hb = 2 * hp, 2 * hp + 1
                        po_a = pso.tile([128, QC], f32, tag="po")
                        po_b = pso.tile([128, QC], f32, tag="po")
                        for kc in range(K):
                            emit_chunk(ha, hb, qc, po_a, po_b, kc, K)
                        evict(ha, qc, po_a)
                        evict(hb, qc, po_b)
                    for mt in range(4 * qc, 4 * qc + 4):
                        c_proj(mt)


def build_program():
    """Build and compile the per-core Bass program (cached)."""
    if "nc" in _CACHE:
        return _CACHE["nc"]
    import concourse.bacc as bacc
    import concourse.tile as tile
    from concourse import mybir

    f32 = mybir.dt.float32
    bf16 = mybir.dt.bfloat16
    nc = bacc.Bacc("TRN2", target_bir_lowering=False, debug=False,
                   num_devices=N_CORES)
    io = {
        "xT": nc.dram_tensor("xT", [C, T], bf16, kind="ExternalInput").ap(),
        "wq": nc.dram_tensor("wq", [C, HL], bf16, kind="ExternalInput").ap(),
        "wk": nc.dram_tensor("wk", [C, HL], bf16, kind="ExternalInput").ap(),
        "wv": nc.dram_tensor("wv", [C, HL], bf16, kind="ExternalInput").ap(),
        "wc": nc.dram_tensor("wc", [HL, C], bf16, kind="ExternalInput").ap(),
        "maskw": nc.dram_tensor("maskw", [128, 2, 128], bf16,
                                kind="ExternalInput").ap(),
        "out": nc.dram_tensor("out", [T, C], bf16, kind="ExternalOutput").ap(),
    }
    with tile.TileContext(nc) as tc:
        _emit(nc, tc, tile, mybir, io)
    nc.compile()
    _CACHE["nc"] = nc
    return nc


def make_in_maps(x, Wq, Wk, Wv, Wc):
    import ml_dtypes
    bf16 = ml_dtypes.bfloat16
    x = np.asarray(x, dtype=np.float32)
    Wq = np.asarray(Wq, dtype=np.float32).astype(bf16)
    Wk = np.asarray(Wk, dtype=np.float32).astype(bf16)
    Wv = np.asarray(Wv, dtype=np.float32).astype(bf16)
    Wc = np.asarray(Wc, dtype=np.float32).astype(bf16)

    # causal triangle for the leading 128-wide diagonal block (after
    # truncating fully-masked columns), duplicated for the head pair
    i_idx = np.arange(128)[:, None]
    j_idx = np.arange(128)[None, :]
    tri = (j_idx >= i_idx).astype(bf16)          # [128, 128]
    maskw = np.repeat(tri[:, None, :], 2, axis=1)  # [128, 2, 128]

    in_maps = []
    for b in range(B):
        xT = np.ascontiguousarray(x[b].T).astype(bf16)
        for g in range(2):
            sl = slice(g * HL, (g + 1) * HL)
            in_maps.append({
                "xT": xT,
                "wq": np.ascontiguousarray(Wq[:, sl]),
                "wk": np.ascontiguousarray(Wk[:, sl]),
                "wv": np.ascontiguousarray(Wv[:, sl]),
                "wc": np.ascontiguousarray(Wc[sl, :]),
                "maskw": maskw,
            })
    return in_maps


def kernel(x, Wq, Wk, Wv, Wc, bc):
    from concourse.bass_utils import run_bass_kernel_spmd

    nc = build_program()
    in_maps = make_in_maps(x, Wq, Wk, Wv, Wc)
    res = run_bass_kernel_spmd(nc, in_maps, core_ids=list(range(N_CORES)))
    bc = np.asarray(bc, dtype=np.float32)
    out = np.empty((B, T, C), dtype=np.float32)
    for b in range(B):
        out[b] = (res.results[2 * b]["out"].astype(np.float32)
                  + res.results[2 * b + 1]["out"].astype(np.float32) + bc)
    return out

